# revision 4
# baseline (speedup 1.0000x reference)
"""ConvAttention Trainium2 kernel.

Per-core (data-parallel over batch, 8 cores, 1 image each):
  q/k/v = depthwise 3x3 conv over x [56,56,64] (+bias), then full
  attention over N=3136 tokens with softmax(q.k * 8), then ctx @ Wp + bp.

Layout strategy (v2):
  - x arrives as f32r; PE-transposed 2 image rows at a time into a
    dual-row stacked padded image xpT2 [128, 58, 58]: partitions 0:64
    hold padded row hh, partitions 64:128 hold row hh+1.  A 3x3 depthwise
    conv then needs only 6 PE passes (3 two-tap K=128 passes + 3 one-tap
    K=64 passes on the upper half) instead of 9.
  - convs run as diagonal-weight matmuls; k and v share one output
    (M=128: k rows 0:64, v rows 64:128), q separate (M=64).
  - scores are computed transposed: s^T[k_token, q_token] so softmax's
    k-reduction is done by the AV matmul itself (ones column in v_nat).
  - exp runs on ACT straight out of PSUM with scale=8.0 (no max pass;
    scores*8 stays far from fp32 overflow).  ACT does nothing else.
  - AV accumulates [ctx^T; rowsum] in PSUM; the projection runs with
    ctx^T as the stationary operand in bf16 (out = token-major directly),
    with an extra wp column that passes rowsum through, so the final
    normalization is a per-partition reciprocal+scale.  No output-side
    PE transposes.
  - all attention matmuls use float32r; weight upload is a single
    concatenated [97,64] tensor (3 DMAs); x is loaded in 4 chunks on the
    sync queue while weights go on the scalar queue.
  - setup (x transposes, k/v convs, v transposes) is emission-interleaved
    with q-tile 0's attention so ACT starts ~6us in.
"""

import sys

import numpy as np

if "/opt/trn_rl_repo" not in sys.path:
    sys.path.insert(0, "/opt/trn_rl_repo")

H = 56
W = 56
C = 64
E = 64
N = H * W               # 3136 tokens
HP = H + 2              # 58 padded rows/cols
NQ = 448                # q-tile (8 spatial rows)
NQT = N // NQ           # 7
KC = 128                # k-chunk (partition dim of s^T tiles)
NKC = (N + KC - 1) // KC  # 25 (last chunk is 64 real tokens)
NPB = (NKC + 1) // 2    # 13 chunk pairs
TCH = 112               # x-transpose chunk = 2 spatial rows
NTC = N // TCH          # 28
TC4 = 112               # final-stage token chunk (4 per q-tile)
NCORES = 8

_CACHE = {}


def _build():
    import concourse.bacc as bacc
    import concourse.tile as tile
    from concourse import mybir
    from concourse.masks import make_identity

    F32 = mybir.dt.float32
    F32R = mybir.dt.float32r
    BF16 = mybir.dt.bfloat16
    AF = mybir.ActivationFunctionType

    nc = bacc.Bacc(None, target_bir_lowering=False, debug=False)

    x_d = nc.dram_tensor("x", [N, C], F32R, kind="ExternalInput")
    wcat_d = nc.dram_tensor("wcat", [97, C], F32, kind="ExternalInput")
    out_d = nc.dram_tensor("out", [N, E], F32, kind="ExternalOutput")

    with tile.TileContext(nc) as tc:
        with tc.tile_pool(name="const", bufs=1) as const, \
             tc.tile_pool(name="big", bufs=1) as big:
            # ---- DMAs first: x on sync queue (4 chunks), weights on scalar
            xstage = big.tile([TCH, NTC, C], F32R)
            xsrc = x_d[:].rearrange("(r p) c -> p r c", p=TCH)
            for dc in range(4):
                nc.sync.dma_start(xstage[:, dc * 7:(dc + 1) * 7, :],
                                  xsrc[:, dc * 7:(dc + 1) * 7, :])
            # wT2: [wq taps 0:9 | wk 9:18 | wv 18:27 | bq 27 | bk 28 | bv 29]
            # transposed to [channel, col]; duplicated on both partition halves
            wT2 = const.tile([128, 30], F32)
            nc.scalar.dma_start(wT2[0:C, :], wcat_d[0:30, :].transpose([1, 0]))
            nc.scalar.dma_start(wT2[C:128, :], wcat_d[0:30, :].transpose([1, 0]))
            wp_f = const.tile([C + 1, E], F32)
            nc.scalar.dma_start(wp_f[:], wcat_d[32:97, :])

            # ---- small constants
            ident_f = const.tile([128, 128], F32)
            make_identity(nc, ident_f[:])                    # Pool
            zsc_f = const.tile([128, 128], F32)
            nc.gpsimd.memset(zsc_f[:], 0.0)                  # Pool
            ones_f = const.tile([128, 32], F32)
            nc.gpsimd.memset(ones_f[:], 1.0)                 # Pool

            ident_r = const.tile([128, 128], F32R)
            nc.vector.tensor_copy(ident_r[:], ident_f[:])    # DVE
            zsc_r = const.tile([128, 128], F32R)
            nc.vector.tensor_copy(zsc_r[:], zsc_f[:])        # DVE
            ones_r = const.tile([128, 32], F32R)
            nc.vector.tensor_copy(ones_r[:], ones_f[:])      # DVE

            # combined k|v bias as per-partition scalars (Pool)
            bkvT = const.tile([128, 1], F32)
            nc.gpsimd.tensor_copy(bkvT[0:C, :], wT2[0:C, 28:29])
            nc.gpsimd.tensor_copy(bkvT[C:128, :], wT2[C:128, 29:30])

            # projection rhs [C+1, E+1] bf16: cols 0:64 = [Wp; bp],
            # col 64 = e_64 (passes rowsum through the matmul)
            wp_bf = const.tile([C + 1, E + 1], BF16)
            nc.gpsimd.tensor_copy(wp_bf[:, 0:E], wp_f[:])
            nc.gpsimd.tensor_copy(wp_bf[0:C, E:E + 1], zsc_f[0:C, 0:1])
            nc.gpsimd.tensor_copy(wp_bf[C:C + 1, E:E + 1], ones_f[C:C + 1, 0:1])

            # ---- conv lhsT tiles: diagonal(w_tap); pairs contract taps
            # (0,j)+(1,j) over K=128, singles do tap (2,j) on partitions 64:128
            kvw_p = const.tile([128, 3, 128], F32R)
            kvw_s = const.tile([128, 3, 128], F32R)
            qw_p = const.tile([128, 3, C], F32R)
            qw_s = const.tile([128, 3, C], F32R)
            idA = ident_f[0:C, 0:C]
            idB = ident_f[C:128, C:128]

            # kv diags on Pool (18 ops), q diags on DVE (9 ops)
            for j in range(3):
                nc.gpsimd.tensor_scalar_mul(kvw_p[0:C, j, 0:C], idA, wT2[0:C, 9 + j:10 + j])
                nc.gpsimd.tensor_scalar_mul(kvw_p[0:C, j, C:128], idA, wT2[0:C, 18 + j:19 + j])
                nc.gpsimd.tensor_scalar_mul(kvw_p[C:128, j, 0:C], idB, wT2[C:128, 12 + j:13 + j])
                nc.gpsimd.tensor_scalar_mul(kvw_p[C:128, j, C:128], idB, wT2[C:128, 21 + j:22 + j])
                nc.gpsimd.tensor_scalar_mul(kvw_s[C:128, j, 0:C], idB, wT2[C:128, 15 + j:16 + j])
                nc.gpsimd.tensor_scalar_mul(kvw_s[C:128, j, C:128], idB, wT2[C:128, 24 + j:25 + j])
                nc.vector.tensor_scalar_mul(qw_p[0:C, j, :], idA, wT2[0:C, 0 + j:1 + j])
                nc.vector.tensor_scalar_mul(qw_p[C:128, j, :], idB, wT2[C:128, 3 + j:4 + j])
                nc.vector.tensor_scalar_mul(qw_s[C:128, j, :], idB, wT2[C:128, 6 + j:7 + j])

            # ---- big persistent tensors
            # xpT2: dual-row stacked padded image.
            #   top  [0:64,  hh, ww] = xp[c, hh,   ww]
            #   bot  [64:128,hh, ww] = xp[c, hh+1, ww]
            # (xp = zero-padded image, rows/cols 0..57; image row h = xp h+1)
            xpT2 = big.tile([128, HP, HP], F32R)
            kvT = big.tile([128, N], F32R)      # rows 0:64 k^T, 64:128 v^T
            qT = big.tile([C, N], F32R)         # q^T [c, token]
            v_nat = big.tile([128, NKC, C + 1], F32R)  # [tok%128, chunk, c|ones]

            # zero borders: top row 0, bottom row 56 (=xp row 57), cols 0 & 57
            nc.vector.tensor_copy(xpT2[0:C, 0, :], zsc_r[0:C, 0:HP])
            nc.vector.tensor_copy(xpT2[C:128, H, :], zsc_r[C:128, 0:HP])
            nc.vector.tensor_copy(xpT2[:, :, 0:1], zsc_r[:, 0:HP].unsqueeze(2))
            nc.vector.tensor_copy(xpT2[:, :, HP - 1:HP], zsc_r[:, 0:HP].unsqueeze(2))
            # ones column for the rowsum trick
            nc.vector.tensor_copy(v_nat[:, :, C], ones_r[:, 0:NKC])

            with tc.tile_pool(name="psT", bufs=1, space="PSUM") as psT, \
                 tc.tile_pool(name="psV", bufs=2, space="PSUM") as psV, \
                 tc.tile_pool(name="psS", bufs=2, space="PSUM") as psS, \
                 tc.tile_pool(name="psC", bufs=1, space="PSUM") as psC, \
                 tc.tile_pool(name="sbA", bufs=3) as sbA, \
                 tc.tile_pool(name="sbB", bufs=2) as sbB, \
                 tc.tile_pool(name="sbF", bufs=2) as sbF:

                def do_xT(t):
                    # transpose 2 image rows; copy into both halves of xpT2
                    pt = psT.tile([C, TCH], F32R, tag="tp")
                    nc.tensor.transpose(pt[:], xstage[:, t, :],
                                        ident_r[0:TCH, 0:TCH])
                    src = pt[:].rearrange("c (h w) -> c h w", w=W)
                    # image rows 2t,2t+1 = xp rows 2t+1,2t+2
                    nc.vector.tensor_copy(
                        xpT2[0:C, 1 + 2 * t:3 + 2 * t, 1:1 + W], src)
                    nc.gpsimd.tensor_copy(
                        xpT2[C:128, 2 * t:2 + 2 * t, 1:1 + W], src)

                def conv6(dst, wp_t, ws_t, h0):
                    # 6-pass depthwise conv into PSUM dst
                    for j in range(3):
                        nc.tensor.matmul(
                            dst, wp_t[:, j, :],
                            xpT2[:, h0:h0 + 8, j:j + W],
                            start=(j == 0), stop=False)
                    for j in range(3):
                        nc.tensor.matmul(
                            dst, ws_t[C:128, j, :],
                            xpT2[C:128, h0 + 1:h0 + 9, j:j + W],
                            start=False, stop=(j == 2))

                def do_kv(ct):
                    pkv = psV.tile([128, NQ], F32, tag="cv")
                    conv6(pkv[:], kvw_p, kvw_s, ct * 8)
                    nc.vector.tensor_scalar_add(
                        kvT[:, ct * NQ:(ct + 1) * NQ], pkv[:], bkvT[:, 0:1])

                def do_vT(kc):
                    cw = min(KC, N - kc * KC)
                    tp = psV.tile([128, C], F32R, tag="cv")
                    nc.tensor.transpose(
                        tp[0:cw, :], kvT[C:128, kc * KC:kc * KC + cw],
                        ident_r[C:128, C:128])
                    if kc % 2 == 0:
                        nc.vector.tensor_copy(v_nat[0:cw, kc, 0:C], tp[0:cw, :])
                    else:
                        nc.gpsimd.tensor_copy(v_nat[0:cw, kc, 0:C], tp[0:cw, :])

                def do_q(qt):
                    ps = psS.tile([128, 2, 512], F32, tag="s")
                    pq = ps[0:C, 0, 0:NQ]
                    conv6(pq, qw_p, qw_s, qt * 8)
                    nc.vector.tensor_scalar_add(
                        qT[:, qt * NQ:(qt + 1) * NQ], pq, wT2[0:C, 27:28])

                # per-tile attention state
                state = {}

                def do_spair(qt, b):
                    q0 = qt * NQ
                    nb = min(2, NKC - b * 2)
                    pw = 64 if (b * 2 + nb) == NKC else 128
                    ps_s = psS.tile([128, 2, 512], F32, tag="s")
                    pT = sbA.tile([128, 2, NQ], F32R, tag="p")
                    for jj in range(nb):
                        kc = b * 2 + jj
                        cw = min(KC, N - kc * KC)
                        nc.tensor.matmul(
                            ps_s[0:cw, jj, 0:NQ],
                            kvT[0:C, kc * KC:kc * KC + cw],
                            qT[:, q0:q0 + NQ],
                            start=True, stop=True)
                    nc.scalar.activation(
                        pT[0:pw, 0:nb, :], ps_s[0:pw, 0:nb, 0:NQ],
                        AF.Exp, scale=8.0)
                    state[(qt, b)] = pT

                def do_av(qt, b):
                    if b == 0:
                        pctx_t = psC.tile([C + 1, NQ], F32, tag="ctx")
                        state["pctx"] = pctx_t
                    pctx = state["pctx"]
                    pT = state.pop((qt, b))
                    nb = min(2, NKC - b * 2)
                    for jj in range(nb):
                        kc = b * 2 + jj
                        cw = min(KC, N - kc * KC)
                        nc.tensor.matmul(
                            pctx[:], v_nat[0:cw, kc, :], pT[0:cw, jj, :],
                            start=(kc == 0), stop=(kc == NKC - 1))

                def do_fin(qt):
                    q0 = qt * NQ
                    pctx = state.pop("pctx")
                    fin = sbF.tile([TC4, 4, E], mybir.dt.float32, tag="fin")
                    for c4 in range(4):
                        ctxT = sbB.tile([C + 1, TC4], BF16, tag="ctxT")
                        nc.vector.tensor_copy(
                            ctxT[:], pctx[:, c4 * TC4:(c4 + 1) * TC4])
                        pf = psV.tile([TC4, E + 1], F32, tag="cv")
                        nc.tensor.matmul(pf[:], ctxT[:], wp_bf[:],
                                         start=True, stop=True)
                        inv = sbB.tile([TC4, 1], mybir.dt.float32, tag="inv")
                        nc.vector.reciprocal(inv[:], pf[:, E:E + 1])
                        nc.gpsimd.tensor_scalar_mul(
                            fin[:, c4, :], pf[:, 0:E], inv[:, 0:1])
                    nc.sync.dma_start(
                        out_d[q0:q0 + NQ, :].rearrange("(c p) e -> p c e", p=TC4),
                        fin[:])

                # ---- interleaved setup + q-tile 0 ----
                # kv(ct) needs xT <= 4ct+4; chunks available after kv(ct):
                # 0..2, 3..6, 7..9, 10..13, 14..16, 17..20, 21..24
                for t in range(0, 5):
                    do_xT(t)
                do_kv(0)
                for t in range(5, 9):
                    do_xT(t)
                do_kv(1)
                for kc in range(0, 7):
                    do_vT(kc)
                do_q(0)
                do_spair(0, 0)
                do_spair(0, 1)
                do_av(0, 0)
                for t in range(9, 13):
                    do_xT(t)
                do_kv(2)
                for kc in range(7, 10):
                    do_vT(kc)
                do_spair(0, 2)
                do_spair(0, 3)
                do_av(0, 1)
                do_av(0, 2)
                for t in range(13, 17):
                    do_xT(t)
                do_kv(3)
                for kc in range(10, 14):
                    do_vT(kc)
                do_spair(0, 4)
                do_spair(0, 5)
                do_av(0, 3)
                do_av(0, 4)
                for t in range(17, 21):
                    do_xT(t)
                do_kv(4)
                for kc in range(14, 17):
                    do_vT(kc)
                do_spair(0, 6)
                do_spair(0, 7)
                do_av(0, 5)
                do_av(0, 6)
                for t in range(21, 25):
                    do_xT(t)
                do_kv(5)
                for kc in range(17, 21):
                    do_vT(kc)
                do_spair(0, 8)
                do_spair(0, 9)
                do_av(0, 7)
                do_av(0, 8)
                for t in range(25, 28):
                    do_xT(t)
                do_kv(6)
                for kc in range(21, 25):
                    do_vT(kc)
                do_spair(0, 10)
                do_spair(0, 11)
                do_av(0, 9)
                do_av(0, 10)
                do_spair(0, 12)
                do_av(0, 11)
                do_av(0, 12)
                do_fin(0)

                # ---- q-tiles 1..6: AVs lag scores by one pair so the PE
                # never waits on ACT
                for qt in range(1, NQT):
                    do_q(qt)
                    do_spair(qt, 0)
                    for b in range(1, NPB):
                        do_spair(qt, b)
                        do_av(qt, b - 1)
                    do_av(qt, NPB - 1)
                    do_fin(qt)

    nc.compile()
    return nc


def _get_nc():
    if "nc" not in _CACHE:
        _CACHE["nc"] = _build()
    return _CACHE["nc"]


def kernel(x, wq, bq, wk, bk, wv, bv, Wp, bp):
    from concourse.bass_utils import run_bass_kernel_spmd

    nc = _get_nc()
    x = np.ascontiguousarray(np.asarray(x, dtype=np.float32))
    wcat = np.zeros((97, C), np.float32)
    wcat[0:9] = np.asarray(wq, np.float32).reshape(9, C)
    wcat[9:18] = np.asarray(wk, np.float32).reshape(9, C)
    wcat[18:27] = np.asarray(wv, np.float32).reshape(9, C)
    wcat[27] = np.asarray(bq, np.float32)
    wcat[28] = np.asarray(bk, np.float32)
    wcat[29] = np.asarray(bv, np.float32)
    wcat[32:96] = np.asarray(Wp, np.float32)
    wcat[96] = np.asarray(bp, np.float32)
    wcat = np.ascontiguousarray(wcat)
    in_maps = [{"x": x[i].reshape(N, C), "wcat": wcat} for i in range(NCORES)]
    res = run_bass_kernel_spmd(nc, in_maps, core_ids=list(range(NCORES)))
    out = np.stack([res.results[i]["out"].reshape(H, W, E) for i in range(NCORES)])
    return out


# revision 10
# speedup vs baseline: 1.0304x; 1.0304x over previous
"""ConvAttention Trainium2 kernel.

Per-core (data-parallel over batch, 8 cores, 1 image each):
  q/k/v = depthwise 3x3 conv over x [56,56,64] (+bias), then full
  attention over N=3136 tokens with softmax(q.k * 8), then ctx @ Wp + bp.

Layout strategy (v2):
  - x arrives as f32r; PE-transposed 2 image rows at a time into a
    dual-row stacked padded image xpT2 [128, 58, 58]: partitions 0:64
    hold padded row hh, partitions 64:128 hold row hh+1.  A 3x3 depthwise
    conv then needs only 6 PE passes (3 two-tap K=128 passes + 3 one-tap
    K=64 passes on the upper half) instead of 9.
  - convs run as diagonal-weight matmuls; k and v share one output
    (M=128: k rows 0:64, v rows 64:128), q separate (M=64).
  - scores are computed transposed: s^T[k_token, q_token] so softmax's
    k-reduction is done by the AV matmul itself (ones column in v_nat).
  - exp runs on ACT straight out of PSUM with scale=8.0 (no max pass;
    scores*8 stays far from fp32 overflow).  ACT does nothing else.
  - AV accumulates [ctx^T; rowsum] in PSUM; the projection runs with
    ctx^T as the stationary operand in bf16 (out = token-major directly),
    with an extra wp column that passes rowsum through, so the final
    normalization is a per-partition reciprocal+scale.  No output-side
    PE transposes.
  - all attention matmuls use float32r; weight upload is a single
    concatenated [97,64] tensor (3 DMAs); x is loaded in 4 chunks on the
    sync queue while weights go on the scalar queue.
  - setup (x transposes, k/v convs, v transposes) is emission-interleaved
    with q-tile 0's attention so ACT starts ~6us in.
"""

import sys

import numpy as np

if "/opt/trn_rl_repo" not in sys.path:
    sys.path.insert(0, "/opt/trn_rl_repo")

H = 56
W = 56
C = 64
E = 64
N = H * W               # 3136 tokens
HP = H + 2              # 58 padded rows/cols
NQ = 448                # q-tile (8 spatial rows)
NQT = N // NQ           # 7
KC = 128                # k-chunk (partition dim of s^T tiles)
NKC = (N + KC - 1) // KC  # 25 (last chunk is 64 real tokens)
NPB = (NKC + 1) // 2    # 13 chunk pairs
TCH = 112               # x-transpose chunk = 2 spatial rows
NTC = N // TCH          # 28
TC4 = 112               # final-stage token chunk (4 per q-tile)
NCORES = 8

_CACHE = {}


def _build():
    import concourse.bacc as bacc
    import concourse.tile as tile
    from concourse import mybir
    from concourse.masks import make_identity

    F32 = mybir.dt.float32
    F32R = mybir.dt.float32r
    BF16 = mybir.dt.bfloat16
    AF = mybir.ActivationFunctionType

    nc = bacc.Bacc(None, target_bir_lowering=False, debug=False)

    x_d = nc.dram_tensor("x", [N, C], F32R, kind="ExternalInput")
    wcat_d = nc.dram_tensor("wcat", [97, C], F32, kind="ExternalInput")
    out_d = nc.dram_tensor("out", [N, E], F32, kind="ExternalOutput")

    with tile.TileContext(nc) as tc:
        with tc.tile_pool(name="const", bufs=1) as const, \
             tc.tile_pool(name="big", bufs=1) as big:
            # ---- DMAs first: x on sync queue (4 chunks), weights on scalar
            xstage = big.tile([TCH, NTC, C], F32R)
            xsrc = x_d[:].rearrange("(r p) c -> p r c", p=TCH)
            for dc in range(4):
                nc.sync.dma_start(xstage[:, dc * 7:(dc + 1) * 7, :],
                                  xsrc[:, dc * 7:(dc + 1) * 7, :])
            # wT2: [wq taps 0:9 | wk 9:18 | wv 18:27 | bq 27 | bk 28 | bv 29]
            # transposed to [channel, col]; duplicated on both partition halves
            wT2 = const.tile([128, 30], F32)
            nc.scalar.dma_start(wT2[0:C, :], wcat_d[0:30, :].transpose([1, 0]))
            nc.scalar.dma_start(wT2[C:128, :], wcat_d[0:30, :].transpose([1, 0]))
            wp_f = const.tile([C + 1, E], F32)
            nc.scalar.dma_start(wp_f[:], wcat_d[32:97, :])

            # ---- small constants
            ident_f = const.tile([128, 128], F32)
            make_identity(nc, ident_f[:])                    # Pool
            zsc_f = const.tile([128, 128], F32)
            nc.gpsimd.memset(zsc_f[:], 0.0)                  # Pool
            ones_f = const.tile([128, 32], F32)
            nc.gpsimd.memset(ones_f[:], 1.0)                 # Pool

            # ident_r first on DVE: it gates the x transposes
            ident_r = const.tile([128, 128], F32R)
            nc.vector.tensor_copy(ident_r[:], ident_f[:])    # DVE

            # ---- conv lhsT tiles: diagonal(w_tap); pairs contract taps
            # (0,j)+(1,j) over K=128, singles do tap (2,j) on partitions 64:128
            kvw_p = const.tile([128, 3, 128], F32R)
            kvw_s = const.tile([128, 3, 128], F32R)
            qw_p = const.tile([128, 3, C], F32R)
            qw_s = const.tile([128, 3, C], F32R)
            idA = ident_f[0:C, 0:C]
            idB = ident_f[C:128, C:128]

            # kv diags on Pool (18 ops) -- they gate the first conv
            for j in range(3):
                nc.gpsimd.tensor_scalar_mul(kvw_p[0:C, j, 0:C], idA, wT2[0:C, 9 + j:10 + j])
                nc.gpsimd.tensor_scalar_mul(kvw_p[0:C, j, C:128], idA, wT2[0:C, 18 + j:19 + j])
                nc.gpsimd.tensor_scalar_mul(kvw_p[C:128, j, 0:C], idB, wT2[C:128, 12 + j:13 + j])
                nc.gpsimd.tensor_scalar_mul(kvw_p[C:128, j, C:128], idB, wT2[C:128, 21 + j:22 + j])
            for j in range(3):
                nc.gpsimd.tensor_scalar_mul(kvw_s[C:128, j, 0:C], idB, wT2[C:128, 15 + j:16 + j])
                nc.gpsimd.tensor_scalar_mul(kvw_s[C:128, j, C:128], idB, wT2[C:128, 24 + j:25 + j])

            # combined k|v bias as per-partition scalars (Pool)
            bkvT = const.tile([128, 1], F32)
            nc.gpsimd.tensor_copy(bkvT[0:C, :], wT2[0:C, 28:29])
            nc.gpsimd.tensor_copy(bkvT[C:128, :], wT2[C:128, 29:30])

            # remaining f32r constants (DVE, after the first transposes kick off)
            zsc_r = const.tile([128, 128], F32R)
            ones_r = const.tile([128, 32], F32R)

            # projection rhs [C+1, E+1] bf16: cols 0:64 = [Wp; bp],
            # col 64 = e_64 (passes rowsum through the matmul).  Deferred to
            # the end of the Pool queue in _emit (only needed by fin(0)).
            wp_bf = const.tile([C + 1, E + 1], BF16)

            # ---- big persistent tensors
            # xpT2: dual-row stacked padded image.
            #   top  [0:64,  hh, ww] = xp[c, hh,   ww]
            #   bot  [64:128,hh, ww] = xp[c, hh+1, ww]
            # (xp = zero-padded image, rows/cols 0..57; image row h = xp h+1)
            xpT2 = big.tile([128, HP, HP], F32R)
            kvT = big.tile([128, N], F32R)      # rows 0:64 k^T, 64:128 v^T
            qT = big.tile([C, N], F32R)         # q^T [c, token]
            v_nat = big.tile([128, NKC, C + 1], F32R)  # [tok%128, chunk, c|ones]

            def emit_consts_late():
                # DVE: zero borders (top row 0, bottom row 56 = xp row 57,
                # cols 0 & 57), ones column, then the q-conv diags
                nc.vector.tensor_copy(zsc_r[:], zsc_f[:])
                nc.vector.tensor_copy(ones_r[:], ones_f[:])
                nc.vector.tensor_copy(xpT2[0:C, 0, :], zsc_r[0:C, 0:HP])
                nc.vector.tensor_copy(xpT2[C:128, H, :], zsc_r[C:128, 0:HP])
                nc.vector.tensor_copy(xpT2[:, :, 0:1], zsc_r[:, 0:HP].unsqueeze(2))
                nc.vector.tensor_copy(xpT2[:, :, HP - 1:HP], zsc_r[:, 0:HP].unsqueeze(2))
                nc.vector.tensor_copy(v_nat[:, :, C], ones_r[:, 0:NKC])
                for j in range(3):
                    nc.vector.tensor_scalar_mul(qw_p[0:C, j, :], idA, wT2[0:C, 0 + j:1 + j])
                    nc.vector.tensor_scalar_mul(qw_p[C:128, j, :], idB, wT2[C:128, 3 + j:4 + j])
                    nc.vector.tensor_scalar_mul(qw_s[C:128, j, :], idB, wT2[C:128, 6 + j:7 + j])

            def emit_wp_bf():
                nc.gpsimd.tensor_copy(wp_bf[:, 0:E], wp_f[:])
                nc.gpsimd.tensor_copy(wp_bf[0:C, E:E + 1], zsc_f[0:C, 0:1])
                nc.gpsimd.tensor_copy(wp_bf[C:C + 1, E:E + 1], ones_f[C:C + 1, 0:1])

            with tc.tile_pool(name="psT", bufs=1, space="PSUM") as psT, \
                 tc.tile_pool(name="psV", bufs=2, space="PSUM") as psV, \
                 tc.tile_pool(name="psS", bufs=2, space="PSUM") as psS, \
                 tc.tile_pool(name="psC", bufs=1, space="PSUM") as psC, \
                 tc.tile_pool(name="sbA", bufs=3) as sbA, \
                 tc.tile_pool(name="sbB", bufs=4) as sbB, \
                 tc.tile_pool(name="sbC", bufs=8) as sbC, \
                 tc.tile_pool(name="sbF", bufs=2) as sbF:

                def do_xT(t):
                    # transpose 2 image rows; copy into both halves of xpT2.
                    # tops on DVE; bottoms on ACT early (idle), Pool later.
                    pt = psT.tile([C, TCH], F32R, tag="tp")
                    nc.tensor.transpose(pt[:], xstage[:, t, :],
                                        ident_r[0:TCH, 0:TCH])
                    src = pt[:].rearrange("c (h w) -> c h w", w=W)
                    # image rows 2t,2t+1 = xp rows 2t+1,2t+2
                    nc.vector.tensor_copy(
                        xpT2[0:C, 1 + 2 * t:3 + 2 * t, 1:1 + W], src)
                    if t < 9:
                        nc.scalar.copy(
                            xpT2[C:128, 2 * t:2 + 2 * t, 1:1 + W], src)
                    else:
                        nc.gpsimd.tensor_copy(
                            xpT2[C:128, 2 * t:2 + 2 * t, 1:1 + W], src)

                def conv6(dst, wp_t, ws_t, h0):
                    # 6-pass depthwise conv into PSUM dst
                    for j in range(3):
                        nc.tensor.matmul(
                            dst, wp_t[:, j, :],
                            xpT2[:, h0:h0 + 8, j:j + W],
                            start=(j == 0), stop=False)
                    for j in range(3):
                        nc.tensor.matmul(
                            dst, ws_t[C:128, j, :],
                            xpT2[C:128, h0 + 1:h0 + 9, j:j + W],
                            start=False, stop=(j == 2))

                def do_kv(ct):
                    pkv = psV.tile([128, NQ], F32, tag="cv")
                    conv6(pkv[:], kvw_p, kvw_s, ct * 8)
                    nc.vector.tensor_scalar_add(
                        kvT[:, ct * NQ:(ct + 1) * NQ], pkv[:], bkvT[:, 0:1])

                def do_vT(kc):
                    cw = min(KC, N - kc * KC)
                    tp = psV.tile([128, C], F32R, tag="cv")
                    nc.tensor.transpose(
                        tp[0:cw, :], kvT[C:128, kc * KC:kc * KC + cw],
                        ident_r[C:128, C:128])
                    if kc % 2 == 0:
                        nc.vector.tensor_copy(v_nat[0:cw, kc, 0:C], tp[0:cw, :])
                    else:
                        nc.gpsimd.tensor_copy(v_nat[0:cw, kc, 0:C], tp[0:cw, :])

                def do_q(qt):
                    ps = psS.tile([128, 2, 512], F32, tag="s")
                    pq = ps[0:C, 0, 0:NQ]
                    conv6(pq, qw_p, qw_s, qt * 8)
                    nc.vector.tensor_scalar_add(
                        qT[:, qt * NQ:(qt + 1) * NQ], pq, wT2[0:C, 27:28])

                # per-tile attention state
                state = {}

                def do_spair(qt, b):
                    q0 = qt * NQ
                    nb = min(2, NKC - b * 2)
                    pw = 64 if (b * 2 + nb) == NKC else 128
                    ps_s = psS.tile([128, 2, 512], F32, tag="s")
                    pT = sbA.tile([128, 2, NQ], F32R, tag="p")
                    for jj in range(nb):
                        kc = b * 2 + jj
                        cw = min(KC, N - kc * KC)
                        nc.tensor.matmul(
                            ps_s[0:cw, jj, 0:NQ],
                            kvT[0:C, kc * KC:kc * KC + cw],
                            qT[:, q0:q0 + NQ],
                            start=True, stop=True)
                    nc.scalar.activation(
                        pT[0:pw, 0:nb, :], ps_s[0:pw, 0:nb, 0:NQ],
                        AF.Exp, scale=8.0)
                    state[(qt, b)] = pT

                def do_av(qt, b):
                    if b == 0:
                        pctx_t = psC.tile([C + 1, NQ], F32, tag="ctx")
                        state["pctx"] = pctx_t
                    pctx = state["pctx"]
                    pT = state.pop((qt, b))
                    nb = min(2, NKC - b * 2)
                    for jj in range(nb):
                        kc = b * 2 + jj
                        cw = min(KC, N - kc * KC)
                        nc.tensor.matmul(
                            pctx[:], v_nat[0:cw, kc, :], pT[0:cw, jj, :],
                            start=(kc == 0), stop=(kc == NKC - 1))

                def do_fin_copy(qt):
                    # DVE-only: drain pctx to SBUF bf16 right after the last AV
                    pctx = state.pop("pctx")
                    ctxs = []
                    for c4 in range(4):
                        ctxT = sbC.tile([C + 1, TC4], BF16, tag="ctxT")
                        nc.vector.tensor_copy(
                            ctxT[:], pctx[:, c4 * TC4:(c4 + 1) * TC4])
                        ctxs.append(ctxT)
                    state[("ctx", qt)] = ctxs

                def do_fin_proj(qt):
                    q0 = qt * NQ
                    ctxs = state.pop(("ctx", qt))
                    fin = sbF.tile([TC4, 4, E], mybir.dt.float32, tag="fin")
                    for c4 in range(4):
                        pf = psV.tile([TC4, E + 1], F32, tag="cv")
                        nc.tensor.matmul(pf[:], ctxs[c4][:], wp_bf[:],
                                         start=True, stop=True)
                        inv = sbB.tile([TC4, 1], mybir.dt.float32, tag="inv")
                        nc.vector.reciprocal(inv[:], pf[:, E:E + 1])
                        nc.gpsimd.tensor_scalar_mul(
                            fin[:, c4, :], pf[:, 0:E], inv[:, 0:1])
                    nc.sync.dma_start(
                        out_d[q0:q0 + NQ, :].rearrange("(c p) e -> p c e", p=TC4),
                        fin[:])

                # ---- interleaved setup + q-tile 0 ----
                # kv(ct) needs xT <= 4ct+4; chunks available after kv(ct):
                # 0..2, 3..6, 7..9, 10..13, 14..16, 17..20, 21..24
                for t in range(0, 5):
                    do_xT(t)
                emit_consts_late()
                do_kv(0)
                do_q(0)
                do_spair(0, 0)
                for t in range(5, 9):
                    do_xT(t)
                do_kv(1)
                for kc in range(0, 7):
                    do_vT(kc)
                do_spair(0, 1)
                do_av(0, 0)
                for t in range(9, 13):
                    do_xT(t)
                do_kv(2)
                for kc in range(7, 10):
                    do_vT(kc)
                do_spair(0, 2)
                do_spair(0, 3)
                do_av(0, 1)
                do_av(0, 2)
                for t in range(13, 17):
                    do_xT(t)
                do_kv(3)
                for kc in range(10, 14):
                    do_vT(kc)
                do_spair(0, 4)
                do_spair(0, 5)
                do_av(0, 3)
                do_av(0, 4)
                for t in range(17, 21):
                    do_xT(t)
                do_kv(4)
                for kc in range(14, 17):
                    do_vT(kc)
                do_spair(0, 6)
                do_spair(0, 7)
                do_av(0, 5)
                do_av(0, 6)
                for t in range(21, 25):
                    do_xT(t)
                do_kv(5)
                for kc in range(17, 21):
                    do_vT(kc)
                emit_wp_bf()
                do_spair(0, 8)
                do_spair(0, 9)
                do_av(0, 7)
                do_av(0, 8)
                for t in range(25, 28):
                    do_xT(t)
                do_kv(6)
                for kc in range(21, 25):
                    do_vT(kc)
                do_spair(0, 10)
                do_spair(0, 11)
                do_av(0, 9)
                do_av(0, 10)
                do_q(1)
                do_spair(0, 12)
                do_av(0, 11)
                do_av(0, 12)
                do_spair(1, 0)
                do_fin_copy(0)

                # ---- q-tiles 1..6: AVs lag scores by one pair so the PE
                # never waits on ACT; the next tile's q conv is hoisted to
                # mid-tile and its first score pair to before the finalize,
                # so ACT never stalls at a tile boundary.
                for qt in range(1, NQT):
                    do_spair(qt, 1)
                    do_av(qt, 0)
                    do_spair(qt, 2)
                    do_av(qt, 1)
                    do_spair(qt, 3)
                    do_av(qt, 2)
                    do_fin_proj(qt - 1)
                    do_spair(qt, 4)
                    do_av(qt, 3)
                    do_spair(qt, 5)
                    do_av(qt, 4)
                    if qt < NQT - 1:
                        do_q(qt + 1)
                    do_spair(qt, 6)
                    do_av(qt, 5)
                    for b in range(7, NPB):
                        do_spair(qt, b)
                        do_av(qt, b - 1)
                    do_av(qt, NPB - 1)
                    if qt < NQT - 1:
                        do_spair(qt + 1, 0)
                    do_fin_copy(qt)
                do_fin_proj(NQT - 1)

    nc.compile()
    return nc


def _get_nc():
    if "nc" not in _CACHE:
        _CACHE["nc"] = _build()
    return _CACHE["nc"]


def kernel(x, wq, bq, wk, bk, wv, bv, Wp, bp):
    from concourse.bass_utils import run_bass_kernel_spmd

    nc = _get_nc()
    x = np.ascontiguousarray(np.asarray(x, dtype=np.float32))
    wcat = np.zeros((97, C), np.float32)
    wcat[0:9] = np.asarray(wq, np.float32).reshape(9, C)
    wcat[9:18] = np.asarray(wk, np.float32).reshape(9, C)
    wcat[18:27] = np.asarray(wv, np.float32).reshape(9, C)
    wcat[27] = np.asarray(bq, np.float32)
    wcat[28] = np.asarray(bk, np.float32)
    wcat[29] = np.asarray(bv, np.float32)
    wcat[32:96] = np.asarray(Wp, np.float32)
    wcat[96] = np.asarray(bp, np.float32)
    wcat = np.ascontiguousarray(wcat)
    in_maps = [{"x": x[i].reshape(N, C), "wcat": wcat} for i in range(NCORES)]
    res = run_bass_kernel_spmd(nc, in_maps, core_ids=list(range(NCORES)))
    out = np.stack([res.results[i]["out"].reshape(H, W, E) for i in range(NCORES)])
    return out


# revision 18
# speedup vs baseline: 1.0978x; 1.0655x over previous
"""ConvAttention Trainium2 kernel.

Per-core (data-parallel over batch, 8 cores, 1 image each):
  q/k/v = depthwise 3x3 conv over x [56,56,64] (+bias), then full
  attention over N=3136 tokens with softmax(q.k * 8), then ctx @ Wp + bp.

Layout strategy (v2):
  - x arrives as f32r; PE-transposed 2 image rows at a time into a
    dual-row stacked padded image xpT2 [128, 58, 58]: partitions 0:64
    hold padded row hh, partitions 64:128 hold row hh+1.  A 3x3 depthwise
    conv then needs only 6 PE passes (3 two-tap K=128 passes + 3 one-tap
    K=64 passes on the upper half) instead of 9.
  - convs run as diagonal-weight matmuls; k and v share one output
    (M=128: k rows 0:64, v rows 64:128), q separate (M=64).
  - scores are computed transposed: s^T[k_token, q_token] so softmax's
    k-reduction is done by the AV matmul itself (ones column in v_nat).
  - exp runs on ACT straight out of PSUM with scale=8.0 (no max pass;
    scores*8 stays far from fp32 overflow).  ACT does nothing else.
  - AV accumulates [ctx^T; rowsum] in PSUM; the projection runs with
    ctx^T as the stationary operand in bf16 (out = token-major directly),
    with an extra wp column that passes rowsum through, so the final
    normalization is a per-partition reciprocal+scale.  No output-side
    PE transposes.
  - all attention matmuls use float32r; weight upload is a single
    concatenated [97,64] tensor (3 DMAs); x is loaded in 4 chunks on the
    sync queue while weights go on the scalar queue.
  - setup (x transposes, k/v convs, v transposes) is emission-interleaved
    with q-tile 0's attention so ACT starts ~6us in.
"""

import sys

import numpy as np

if "/opt/trn_rl_repo" not in sys.path:
    sys.path.insert(0, "/opt/trn_rl_repo")

H = 56
W = 56
C = 64
E = 64
N = H * W               # 3136 tokens
HP = H + 2              # 58 padded rows/cols
NQ = 448                # q-tile (8 spatial rows)
NQT = N // NQ           # 7
KC = 128                # k-chunk (partition dim of s^T tiles)
NKC = (N + KC - 1) // KC  # 25 (last chunk is 64 real tokens)
NPB = (NKC + 1) // 2    # 13 chunk pairs
TCH = 112               # x-transpose chunk = 2 spatial rows
NTC = N // TCH          # 28
TC4 = 112               # final-stage token chunk (4 per q-tile)
NCORES = 8

_CACHE = {}


def _build():
    import concourse.bacc as bacc
    import concourse.tile as tile
    from concourse import mybir
    from concourse.masks import make_identity

    F32 = mybir.dt.float32
    F32R = mybir.dt.float32r
    BF16 = mybir.dt.bfloat16
    AF = mybir.ActivationFunctionType

    nc = bacc.Bacc(None, target_bir_lowering=False, debug=False)

    x_d = nc.dram_tensor("x", [N, C], F32R, kind="ExternalInput")
    wcat_d = nc.dram_tensor("wcat", [97, C], F32, kind="ExternalInput")
    out_d = nc.dram_tensor("out", [N, E], F32, kind="ExternalOutput")

    with tile.TileContext(nc) as tc:
        with tc.tile_pool(name="const", bufs=1) as const, \
             tc.tile_pool(name="big", bufs=1) as big:
            # ---- DMAs first: x on sync queue (4 chunks), weights on scalar
            xstage = big.tile([TCH, NTC, C], F32R)
            xsrc = x_d[:].rearrange("(r p) c -> p r c", p=TCH)
            for dc in range(4):
                nc.sync.dma_start(xstage[:, dc * 7:(dc + 1) * 7, :],
                                  xsrc[:, dc * 7:(dc + 1) * 7, :])
            # wT2: [wq taps 0:9 | wk 9:18 | wv 18:27 | bq 27 | bk 28 | bv 29]
            # transposed to [channel, col]; duplicated on both partition halves
            wT2 = const.tile([128, 30], F32)
            nc.scalar.dma_start(wT2[0:C, :], wcat_d[0:30, :].transpose([1, 0]))
            nc.scalar.dma_start(wT2[C:128, :], wcat_d[0:30, :].transpose([1, 0]))
            wp_f = const.tile([C + 1, E], F32)
            nc.scalar.dma_start(wp_f[:], wcat_d[32:97, :])

            # ---- small constants
            ident_f = const.tile([128, 128], F32)
            make_identity(nc, ident_f[:])                    # Pool
            zsc_f = const.tile([128, 128], F32)
            nc.vector.memset(zsc_f[:], 0.0)                  # DVE
            ones_f = const.tile([128, 32], F32)
            nc.vector.memset(ones_f[:], 1.0)                 # DVE

            # ident_r first on DVE: it gates the x transposes
            ident_r = const.tile([128, 128], F32R)
            nc.vector.tensor_copy(ident_r[:], ident_f[:])    # DVE

            # ---- conv lhsT tiles: diagonal(w_tap); pairs contract taps
            # (0,j)+(1,j) over K=128, singles do tap (2,j) on partitions 64:128
            kvw_p = const.tile([128, 3, 128], F32R)
            kvw_s = const.tile([128, 3, 128], F32R)
            qw_p = const.tile([128, 3, C], F32R)
            qw_s = const.tile([128, 3, C], F32R)
            idA = ident_f[0:C, 0:C]
            idB = ident_f[C:128, C:128]

            # kv diags on Pool (18 ops) -- they gate the first conv
            for j in range(3):
                nc.gpsimd.tensor_scalar_mul(kvw_p[0:C, j, 0:C], idA, wT2[0:C, 9 + j:10 + j])
                nc.gpsimd.tensor_scalar_mul(kvw_p[0:C, j, C:128], idA, wT2[0:C, 18 + j:19 + j])
                nc.gpsimd.tensor_scalar_mul(kvw_p[C:128, j, 0:C], idB, wT2[C:128, 12 + j:13 + j])
                nc.gpsimd.tensor_scalar_mul(kvw_p[C:128, j, C:128], idB, wT2[C:128, 21 + j:22 + j])
            for j in range(3):
                nc.gpsimd.tensor_scalar_mul(kvw_s[C:128, j, 0:C], idB, wT2[C:128, 15 + j:16 + j])
                nc.gpsimd.tensor_scalar_mul(kvw_s[C:128, j, C:128], idB, wT2[C:128, 24 + j:25 + j])

            # combined k|v bias as per-partition scalars (Pool)
            bkvT = const.tile([128, 1], F32)
            nc.gpsimd.tensor_copy(bkvT[0:C, :], wT2[0:C, 28:29])
            nc.gpsimd.tensor_copy(bkvT[C:128, :], wT2[C:128, 29:30])

            # remaining f32r constants (DVE, after the first transposes kick off)
            zsc_r = const.tile([128, 128], F32R)
            ones_r = const.tile([128, 32], F32R)

            # projection rhs [C+1, E+1] bf16: cols 0:64 = [Wp; bp],
            # col 64 = e_64 (passes rowsum through the matmul).  Deferred to
            # the end of the Pool queue in _emit (only needed by fin(0)).
            wp_bf = const.tile([C + 1, E + 1], BF16)

            # ---- big persistent tensors
            # xpT2: dual-row stacked padded image.
            #   top  [0:64,  hh, ww] = xp[c, hh,   ww]
            #   bot  [64:128,hh, ww] = xp[c, hh+1, ww]
            # (xp = zero-padded image, rows/cols 0..57; image row h = xp h+1)
            xpT2 = big.tile([128, HP, HP], F32R)
            kvT = big.tile([128, N], F32R)      # rows 0:64 k^T, 64:128 v^T
            qT = big.tile([C, N], F32R)         # q^T [c, token]
            v_nat = big.tile([128, NKC, C + 1], F32R)  # [tok%128, chunk, c|ones]

            def emit_consts_late():
                # DVE: zero borders (top row 0, bottom row 56 = xp row 57,
                # cols 0 & 57), ones column, then the q-conv diags
                nc.vector.tensor_copy(zsc_r[:], zsc_f[:])
                nc.vector.tensor_copy(ones_r[:], ones_f[:])
                nc.vector.tensor_copy(xpT2[0:C, 0, :], zsc_r[0:C, 0:HP])
                nc.vector.tensor_copy(xpT2[C:128, H, :], zsc_r[C:128, 0:HP])
                nc.vector.tensor_copy(xpT2[:, :, 0:1], zsc_r[:, 0:HP].unsqueeze(2))
                nc.vector.tensor_copy(xpT2[:, :, HP - 1:HP], zsc_r[:, 0:HP].unsqueeze(2))
                nc.vector.tensor_copy(v_nat[:, :, C], ones_r[:, 0:NKC])
                for j in range(3):
                    nc.vector.tensor_scalar_mul(qw_p[0:C, j, :], idA, wT2[0:C, 0 + j:1 + j])
                    nc.vector.tensor_scalar_mul(qw_p[C:128, j, :], idB, wT2[C:128, 3 + j:4 + j])
                    nc.vector.tensor_scalar_mul(qw_s[C:128, j, :], idB, wT2[C:128, 6 + j:7 + j])

            def emit_wp_bf():
                nc.gpsimd.tensor_copy(wp_bf[:, 0:E], wp_f[:])
                nc.gpsimd.tensor_copy(wp_bf[0:C, E:E + 1], zsc_f[0:C, 0:1])
                nc.gpsimd.tensor_copy(wp_bf[C:C + 1, E:E + 1], ones_f[C:C + 1, 0:1])

            with tc.tile_pool(name="psT", bufs=2, space="PSUM") as psT, \
                 tc.tile_pool(name="psV", bufs=1, space="PSUM") as psV, \
                 tc.tile_pool(name="psS", bufs=2, space="PSUM") as psS, \
                 tc.tile_pool(name="psC", bufs=1, space="PSUM") as psC, \
                 tc.tile_pool(name="sbA", bufs=3) as sbA, \
                 tc.tile_pool(name="sbB", bufs=4) as sbB, \
                 tc.tile_pool(name="sbC", bufs=8) as sbC, \
                 tc.tile_pool(name="sbF", bufs=2) as sbF:

                def do_xT(t):
                    # transpose 2 image rows; copy into both halves of xpT2.
                    # tops on DVE; bottoms on ACT early (idle), Pool later.
                    pt = psT.tile([C, TCH], F32R, tag="tp")
                    nc.tensor.transpose(pt[:], xstage[:, t, :],
                                        ident_r[0:TCH, 0:TCH])
                    src = pt[:].rearrange("c (h w) -> c h w", w=W)
                    # image rows 2t,2t+1 = xp rows 2t+1,2t+2
                    nc.vector.tensor_copy(
                        xpT2[0:C, 1 + 2 * t:3 + 2 * t, 1:1 + W], src)
                    if t < 5:
                        nc.scalar.copy(
                            xpT2[C:128, 2 * t:2 + 2 * t, 1:1 + W], src)
                    else:
                        nc.gpsimd.tensor_copy(
                            xpT2[C:128, 2 * t:2 + 2 * t, 1:1 + W], src)

                def conv6(dst, wp_t, ws_t, h0):
                    # 6-pass depthwise conv into PSUM dst
                    for j in range(3):
                        nc.tensor.matmul(
                            dst, wp_t[:, j, :],
                            xpT2[:, h0:h0 + 8, j:j + W],
                            start=(j == 0), stop=False)
                    for j in range(3):
                        nc.tensor.matmul(
                            dst, ws_t[C:128, j, :],
                            xpT2[C:128, h0 + 1:h0 + 9, j:j + W],
                            start=False, stop=(j == 2))

                def do_kv(ct):
                    pkv = psV.tile([128, NQ], F32, tag="cv")
                    conv6(pkv[:], kvw_p, kvw_s, ct * 8)
                    nc.vector.tensor_scalar_add(
                        kvT[:, ct * NQ:(ct + 1) * NQ], pkv[:], bkvT[:, 0:1])

                def do_vT(kc):
                    cw = min(KC, N - kc * KC)
                    tp = psT.tile([128, C], F32R, tag="tp")
                    nc.tensor.transpose(
                        tp[0:cw, :], kvT[C:128, kc * KC:kc * KC + cw],
                        ident_r[C:128, C:128])
                    if kc % 2 == 0:
                        nc.vector.tensor_copy(v_nat[0:cw, kc, 0:C], tp[0:cw, :])
                    else:
                        nc.gpsimd.tensor_copy(v_nat[0:cw, kc, 0:C], tp[0:cw, :])

                def do_q(qt, half=None):
                    # q conv in the small psT pool; optionally emitted in two
                    # 3-pass halves so the PE never blocks the score stream
                    # for more than ~0.6us at a time
                    h0 = qt * 8
                    if half in (None, 0):
                        pq_t = psT.tile([C, NQ], F32, tag="tp")
                        state["pq"] = pq_t
                        for j in range(3):
                            nc.tensor.matmul(
                                pq_t[:], qw_p[:, j, :],
                                xpT2[:, h0:h0 + 8, j:j + W],
                                start=(j == 0), stop=False)
                    if half in (None, 1):
                        pq_t = state.pop("pq")
                        for j in range(3):
                            nc.tensor.matmul(
                                pq_t[:], qw_s[C:128, j, :],
                                xpT2[C:128, h0 + 1:h0 + 9, j:j + W],
                                start=False, stop=(j == 2))
                        nc.vector.tensor_scalar_add(
                            qT[:, qt * NQ:(qt + 1) * NQ], pq_t[:], wT2[0:C, 27:28])

                # per-tile attention state
                state = {}

                def do_spair(qt, b):
                    q0 = qt * NQ
                    nb = min(2, NKC - b * 2)
                    pw = 64 if (b * 2 + nb) == NKC else 128
                    ps_s = psS.tile([128, 2, 512], F32, tag="s")
                    pT = sbA.tile([128, 2, NQ], F32R, tag="p")
                    for jj in range(nb):
                        kc = b * 2 + jj
                        cw = min(KC, N - kc * KC)
                        nc.tensor.matmul(
                            ps_s[0:cw, jj, 0:NQ],
                            kvT[0:C, kc * KC:kc * KC + cw],
                            qT[:, q0:q0 + NQ],
                            start=True, stop=True)
                    nc.scalar.activation(
                        pT[0:pw, 0:nb, :], ps_s[0:pw, 0:nb, 0:NQ],
                        AF.Exp, scale=8.0)
                    state[(qt, b)] = pT

                def do_av(qt, b):
                    if b == 0:
                        pctx_t = psC.tile([C + 1, NQ], F32, tag="ctx")
                        state["pctx"] = pctx_t
                    pctx = state["pctx"]
                    pT = state.pop((qt, b))
                    nb = min(2, NKC - b * 2)
                    for jj in range(nb):
                        kc = b * 2 + jj
                        cw = min(KC, N - kc * KC)
                        nc.tensor.matmul(
                            pctx[:], v_nat[0:cw, kc, :], pT[0:cw, jj, :],
                            start=(kc == 0), stop=(kc == NKC - 1))

                def do_fin_copy(qt):
                    # DVE-only: drain pctx to SBUF bf16 right after the last AV
                    pctx = state.pop("pctx")
                    ctxs = []
                    for c4 in range(4):
                        ctxT = sbC.tile([C + 1, TC4], BF16, tag="ctxT")
                        nc.vector.tensor_copy(
                            ctxT[:], pctx[:, c4 * TC4:(c4 + 1) * TC4])
                        ctxs.append(ctxT)
                    state[("ctx", qt)] = ctxs

                def do_fin_proj(qt):
                    q0 = qt * NQ
                    ctxs = state.pop(("ctx", qt))
                    fin = sbF.tile([TC4, 4, E], mybir.dt.float32, tag="fin")
                    for c4 in range(4):
                        pf = psV.tile([TC4, E + 1], F32, tag="cv")
                        nc.tensor.matmul(pf[:], ctxs[c4][:], wp_bf[:],
                                         start=True, stop=True)
                        inv = sbB.tile([TC4, 1], mybir.dt.float32, tag="inv")
                        nc.vector.reciprocal(inv[:], pf[:, E:E + 1])
                        nc.gpsimd.tensor_scalar_mul(
                            fin[:, c4, :], pf[:, 0:E], inv[:, 0:1])
                    nc.sync.dma_start(
                        out_d[q0:q0 + NQ, :].rearrange("(c p) e -> p c e", p=TC4),
                        fin[:])

                # ---- interleaved setup + q-tile 0 ----
                # kv(ct) needs xT <= 4ct+4; chunks available after kv(ct):
                # 0..2, 3..6, 7..9, 10..13, 14..16, 17..20, 21..24
                for t in range(0, 5):
                    do_xT(t)
                emit_consts_late()
                do_kv(0)
                do_q(0)
                do_spair(0, 0)
                for t in range(5, 9):
                    do_xT(t)
                do_kv(1)
                for kc in range(0, 7):
                    do_vT(kc)
                do_spair(0, 1)
                do_av(0, 0)
                for t in range(9, 13):
                    do_xT(t)
                do_kv(2)
                for kc in range(7, 10):
                    do_vT(kc)
                do_spair(0, 2)
                do_spair(0, 3)
                do_av(0, 1)
                do_av(0, 2)
                for t in range(13, 17):
                    do_xT(t)
                do_kv(3)
                for kc in range(10, 14):
                    do_vT(kc)
                do_spair(0, 4)
                do_spair(0, 5)
                do_av(0, 3)
                do_av(0, 4)
                for t in range(17, 21):
                    do_xT(t)
                do_kv(4)
                for kc in range(14, 17):
                    do_vT(kc)
                do_spair(0, 6)
                do_spair(0, 7)
                do_av(0, 5)
                do_av(0, 6)
                for t in range(21, 25):
                    do_xT(t)
                do_kv(5)
                for kc in range(17, 21):
                    do_vT(kc)
                emit_wp_bf()
                do_spair(0, 8)
                do_spair(0, 9)
                do_av(0, 7)
                do_av(0, 8)
                for t in range(25, 28):
                    do_xT(t)
                do_kv(6)
                for kc in range(21, 25):
                    do_vT(kc)
                do_spair(0, 10)
                do_spair(0, 11)
                do_av(0, 9)
                do_av(0, 10)
                do_q(1, half=0)
                do_q(1, half=1)
                do_spair(0, 12)
                do_av(0, 11)
                do_spair(1, 0)
                do_av(0, 12)
                do_fin_copy(0)

                # ---- q-tiles 1..6: AVs lag scores by one pair so the PE
                # never waits on ACT; the next tile's q conv is hoisted to
                # mid-tile and its first score pair to before the finalize,
                # so ACT never stalls at a tile boundary.
                for qt in range(1, NQT):
                    do_spair(qt, 1)
                    do_av(qt, 0)
                    do_spair(qt, 2)
                    do_av(qt, 1)
                    do_spair(qt, 3)
                    do_av(qt, 2)
                    do_fin_proj(qt - 1)
                    do_spair(qt, 4)
                    do_av(qt, 3)
                    if qt < NQT - 1:
                        do_q(qt + 1, half=0)
                    do_spair(qt, 5)
                    do_av(qt, 4)
                    if qt < NQT - 1:
                        do_q(qt + 1, half=1)
                    do_spair(qt, 6)
                    do_av(qt, 5)
                    for b in range(7, NPB):
                        do_spair(qt, b)
                        do_av(qt, b - 1)
                    if qt < NQT - 1:
                        do_spair(qt + 1, 0)
                    do_av(qt, NPB - 1)
                    do_fin_copy(qt)
                do_fin_proj(NQT - 1)

    nc.compile()
    return nc


def _get_nc():
    if "nc" not in _CACHE:
        _CACHE["nc"] = _build()
    return _CACHE["nc"]


def kernel(x, wq, bq, wk, bk, wv, bv, Wp, bp):
    from concourse.bass_utils import run_bass_kernel_spmd

    nc = _get_nc()
    x = np.ascontiguousarray(np.asarray(x, dtype=np.float32))
    wcat = np.zeros((97, C), np.float32)
    wcat[0:9] = np.asarray(wq, np.float32).reshape(9, C)
    wcat[9:18] = np.asarray(wk, np.float32).reshape(9, C)
    wcat[18:27] = np.asarray(wv, np.float32).reshape(9, C)
    wcat[27] = np.asarray(bq, np.float32)
    wcat[28] = np.asarray(bk, np.float32)
    wcat[29] = np.asarray(bv, np.float32)
    wcat[32:96] = np.asarray(Wp, np.float32)
    wcat[96] = np.asarray(bp, np.float32)
    wcat = np.ascontiguousarray(wcat)
    in_maps = [{"x": x[i].reshape(N, C), "wcat": wcat} for i in range(NCORES)]
    res = run_bass_kernel_spmd(nc, in_maps, core_ids=list(range(NCORES)))
    out = np.stack([res.results[i]["out"].reshape(H, W, E) for i in range(NCORES)])
    return out
